# revision 21
# baseline (speedup 1.0000x reference)
"""Causal self-attention (B=4, T=2048, D=1024, H=16) on 8 TRN2 NeuronCores.

Sharding: tensor-parallel over 4 head-groups x data-parallel over 2 batch-groups.
Core c handles batches [2*(c//4), 2*(c//4)+2) and heads [4*(c%4), 4*(c%4)+4).
Each core computes a partial output projection (its 256 feature rows of W_proj);
the host sums the 4 head-group partials per batch group.

Design notes (v3):
- All matmul operands bf16 (PSUM accumulation fp32). Enables Fast Weight Load
  (LDWEIGHTS 97ns -> fully hidden under 216ns matmul streams) and halves SBUF
  traffic. rel-err lands ~4e-3 against the 2e-2 budget.
- x is transposed/packed on the host; x^T tiles DMA straight into SBUF.
- S^T uses 2x row tiling: each head contracts over only 64 dims, so the two
  heads of a packed Q^T/K^T pair run concurrently in rows 0-63 / 64-127 of the
  PE array (tile_position (0,0)/(64,0)), writing the two bank-halves of one
  [128,1024] PSUM tile; exp covers both heads in one ACT op.
- The V stationary block for (key-tile, head) is [64 ones cols | 64 V dims]:
  the PV matmul emits the softmax denominator pre-broadcast into PSUM rows
  0-63 for free (ones cols must map to base partition 0: the custom-DVE
  reciprocal ignores a nonzero input base partition).
- Softmax skips max-subtraction (scores ~N(0,1)) so exp never overflows.
- The QKV production of the NEXT 512-token chunk / next batch is interleaved
  into the attention inner loops as work units drained between PV pairs: the
  attention S-phase is exp(ACT)-paced, so QKV chain matmuls fill the PE gaps.
  QKV chain PSUM tiles and projection PSUM tiles share one 2-bank pool;
  attention needs 6 more (4 for double-buffered S^T pair tiles, 2 for PV
  accumulators) = exactly the 8 PSUM banks.
"""
import functools
from contextlib import ExitStack

import numpy as np
import ml_dtypes

import concourse.bacc as bacc
import concourse.tile as tile
import concourse.mybir as mybir
from concourse.bass_utils import run_bass_kernel_spmd

F32 = mybir.dt.float32
BF16 = mybir.dt.bfloat16
EXP = mybir.ActivationFunctionType.Exp

B, T, D, H, HD = 4, 2048, 1024, 16, 64
NB, NH = 2, 4            # batches / heads per core
NC = 8
NT5 = T // 512           # 4  (512-token chunks)
NTT = T // 128           # 16 (128-token key tiles)
NDK = D // 128           # 8  (feature chunks of input dim)
WCOL = 768               # per-dk weight columns: Q(256) K(256) V(256)


@functools.lru_cache(maxsize=1)
def build():
    nc = bacc.Bacc("TRN2", target_bir_lowering=False, debug=False, num_devices=NC)
    xt_d = nc.dram_tensor("xt", [NB, NT5, NDK, 128, 512], BF16,
                          kind="ExternalInput").ap()
    wqkv_d = nc.dram_tensor("wqkv", [128, NDK * WCOL], BF16,
                            kind="ExternalInput").ap()
    wproj_d = nc.dram_tensor("wproj", [128, 2 * D], BF16,
                             kind="ExternalInput").ap()
    tri_d = nc.dram_tensor("tri", [128, 128], BF16, kind="ExternalInput").ap()
    out_d = nc.dram_tensor("out", [NB, T, D], BF16, kind="ExternalOutput").ap()

    with tile.TileContext(nc) as tc, ExitStack() as ctx:
        const = ctx.enter_context(tc.tile_pool(name="const", bufs=1))
        wpool = ctx.enter_context(tc.tile_pool(name="w", bufs=1))
        actv = ctx.enter_context(tc.tile_pool(name="actv", bufs=1))
        xin_pool = ctx.enter_context(tc.tile_pool(name="xin", bufs=3))
        pP = ctx.enter_context(tc.tile_pool(name="pP", bufs=12))
        ytp = ctx.enter_context(tc.tile_pool(name="ytp", bufs=2))
        ost_pool = ctx.enter_context(tc.tile_pool(name="ost", bufs=2))
        rbp = ctx.enter_context(tc.tile_pool(name="rbp", bufs=2))
        # PSUM: pao (QKV chains + proj) 2 banks, psS 4 banks, psY 2 banks
        pao = ctx.enter_context(tc.tile_pool(name="pao", bufs=2, space="PSUM"))
        psS_pool = ctx.enter_context(
            tc.tile_pool(name="psS", bufs=2, space="PSUM"))
        psY_pool = ctx.enter_context(
            tc.tile_pool(name="psY", bufs=1, space="PSUM"))

        w_sb = wpool.tile([128, NDK * WCOL], BF16)
        wv8 = w_sb.rearrange("p (a c) -> p a c", a=NDK)
        wp_sb = wpool.tile([128, 2 * D], BF16)
        tri = const.tile([128, 128], BF16)          # tri[k,q] = 1.0 iff q >= k

        # per-batch double-buffered activation tiles
        qts = [[actv.tile([128, T], BF16, tag=f"qt{cc}", name=f"qt{cc}_{b}",
                          bufs=2) for cc in range(2)] for b in range(NB)]
        kts = [[actv.tile([128, T], BF16, tag=f"kt{cc}", name=f"kt{cc}_{b}",
                          bufs=2) for cc in range(2)] for b in range(NB)]
        vsbs = [actv.tile([128, NTT * NH * 128], BF16, tag="v", name=f"v_{b}",
                          bufs=2) for b in range(NB)]
        v128s = [v.rearrange("p (n c) -> p n c", c=128) for v in vsbs]

        # ---------- QKV production work units ----------
        # unit = (b, t5, thunk). B(b, j) requires all units with marker
        # (b', t5') <= (b, j) drained; the rest drain opportunistically in
        # attention PE gaps.
        def mk_dma_unit(b, t5):
            def f():
                xa = xin_pool.tile([128, NDK * 512], BF16, tag="xa",
                                   name=f"xa{b}_{t5}")
                xav = xa.rearrange("p (a c) -> p a c", a=NDK)
                for dk in range(NDK):
                    nc.sync.dma_start(xav[:, dk], xt_d[b, t5, dk])
                xas[(b, t5)] = xa
                if t5 == 0:
                    # ones for the denominator cols; split so the first V
                    # evacuation only waits on the first quarter
                    for q in range(4):
                        nc.gpsimd.memset(
                            vsbs[b][:, q * 2048:(q + 1) * 2048], 1.0)
            return f

        def mk_qk_unit(b, t5, kind, cc):
            def f():
                xa = xas[(b, t5)]
                dst = (qts if kind == 0 else kts)[b][cc]
                ps = pao.tile([128, 512], F32, tag="ps",
                              name=f"{'qk'[kind]}{b}{t5}{cc}")
                base = 256 * kind + cc * 128
                for dk in range(NDK):
                    nc.tensor.matmul(
                        ps[:],
                        w_sb[:, dk * WCOL + base:dk * WCOL + base + 128],
                        xa[:, dk * 512:dk * 512 + 512],
                        start=(dk == 0), stop=(dk == NDK - 1))
                nc.vector.tensor_copy(dst[:, t5 * 512:t5 * 512 + 512], ps[:])
            return f

        def mk_v_unit(b, t5, tt):
            def f():
                xa = xas[(b, t5)]
                ps = pao.tile([128, 256], F32, tag="ps", name=f"v{b}{t5}{tt}")
                for dk in range(NDK):
                    nc.tensor.matmul(
                        ps[:],
                        xa[:, dk * 512 + tt * 128:dk * 512 + tt * 128 + 128],
                        w_sb[:, dk * WCOL + 512:dk * WCOL + 768],
                        start=(dk == 0), stop=(dk == NDK - 1))
                ti = t5 * 4 + tt
                nc.vector.tensor_copy(
                    v128s[b][:, ti * NH:ti * NH + NH, 64:128],
                    ps[:].rearrange("p (n c) -> p n c", c=64))
            return f

        xas = {}
        units = []
        for b in range(NB):
            for t5 in range(NT5):
                units.append((b, t5, mk_dma_unit(b, t5)))
                for cc in range(2):
                    units.append((b, t5, mk_qk_unit(b, t5, 0, cc)))
                for cc in range(2):
                    units.append((b, t5, mk_qk_unit(b, t5, 1, cc)))
                for tt in range(4):
                    units.append((b, t5, mk_v_unit(b, t5, tt)))

        state = {"u": 0}

        def drain_until(b, t5):
            while state["u"] < len(units):
                ub, ut5, f = units[state["u"]]
                if (ub, ut5) > (b, t5):
                    return
                f()
                state["u"] += 1

        def drain_one():
            if state["u"] < len(units):
                units[state["u"]][2]()
                state["u"] += 1

        # input DMAs: descriptor generation costs ~0.6us per DMA per queue,
        # so spread the startup DMAs across idle queue engines; weight chunk
        # dk=0 goes first so the first QKV chain can start ASAP
        wqv8 = wqkv_d.rearrange("p (a c) -> p a c", a=NDK)
        for dk in range(NDK):
            eng = (nc.scalar, nc.gpsimd)[dk % 2]
            eng.dma_start(wv8[:, dk], wqv8[:, dk])
        units[0][2]()
        state["u"] = 1
        nc.scalar.dma_start(tri[:], tri_d)
        nc.gpsimd.dma_start(wp_sb[:], wproj_d)

        # ---------- main loop: attention + projection ----------
        for b in range(NB):
            for j in range(NT5):
                drain_until(b, j)
                nk = 4 * j + 4
                offs = [128 * (i - 4 * j) if i > 4 * j else 0
                        for i in range(nk)]
                yt = [ytp.tile([128, 512], BF16, tag=f"yt{ff}",
                               name=f"yt{ff}_{b}{j}") for ff in range(2)]
                for cc in range(2):
                    qt, kt = qts[b][cc], kts[b][cc]
                    psY = [psY_pool.tile([128, 512], F32, tag=f"y{hh}",
                                         name=f"psY{hh}") for hh in range(2)]
                    Ps = []

                    def emit_pv(i):
                        off = offs[i]
                        for hh in range(2):
                            nc.tensor.matmul(
                                psY[hh][:, off:512],
                                v128s[b][:, i * NH + 2 * cc + hh, :],
                                Ps[i][:, hh * 512 + off:hh * 512 + 512],
                                start=(i == 0), stop=(i == nk - 1))

                    for i in range(nk):
                        off = offs[i]
                        s = psS_pool.tile([128, 1024], F32, tag="s",
                                          name=f"s{i % 2}")
                        # two heads concurrently via 2x row tiling
                        nc.tensor.matmul(
                            s[:, off:512],
                            kt[0:64, 128 * i:128 * i + 128],
                            qt[0:64, 512 * j + off:512 * (j + 1)],
                            start=True, stop=True, tile_position=(0, 0))
                        nc.tensor.matmul(
                            s[:, 512 + off:1024],
                            kt[64:128, 128 * i:128 * i + 128],
                            qt[64:128, 512 * j + off:512 * (j + 1)],
                            start=True, stop=True, tile_position=(64, 0))
                        P = pP.tile([128, 1024], BF16, tag="p",
                                    name=f"P{i % 8}")
                        if off == 0:
                            nc.scalar.activation(P[:], s[:], EXP, scale=0.125)
                        else:
                            nc.scalar.activation(P[:, off:512], s[:, off:512],
                                                 EXP, scale=0.125)
                            nc.scalar.activation(P[:, 512 + off:1024],
                                                 s[:, 512 + off:1024],
                                                 EXP, scale=0.125)
                        if i >= 4 * j:      # causal diagonal tile mask
                            nc.vector.tensor_mul(
                                P[:, off:off + 128],
                                P[:, off:off + 128], tri[:])
                            nc.vector.tensor_mul(
                                P[:, 512 + off:512 + off + 128],
                                P[:, 512 + off:512 + off + 128], tri[:])
                        Ps.append(P)
                        # chase S^T with PV 4 key-tiles behind (paired to
                        # halve PE tiling-mode switches), and fill remaining
                        # PE slack with a QKV work unit
                        if i >= 4 and i % 2 == 0:
                            emit_pv(i - 4)
                            emit_pv(i - 3)
                        drain_one()
                    for i in range(max(0, nk - 4), nk):
                        emit_pv(i)
                    # normalize: PSUM rows 0-63 hold the denominator
                    # (broadcast by the all-ones V cols), rows 64-127 = y^T
                    for hh in range(2):
                        rb = rbp.tile([64, 512], F32, tag=f"rb{hh}",
                                      name=f"rb{hh}")
                        nc.vector.reciprocal_approx_fast(
                            rb[:], psY[hh][0:64, :])
                        nc.vector.tensor_mul(
                            yt[cc][64 * hh:64 * hh + 64, :],
                            psY[hh][64:128, :], rb[:])
                    drain_one()
                # ---- output projection for this 512-token block ----
                for g in range(4):
                    ostage = ost_pool.tile([128, D], BF16, tag="og",
                                           name=f"og{g % 2}")
                    for nn2 in range(2):
                        o = pao.tile([128, 512], F32, tag="ps", name=f"o{nn2}")
                        for ff in range(2):
                            nc.tensor.matmul(
                                o[:],
                                yt[ff][:, 128 * g:128 * g + 128],
                                wp_sb[:, ff * D + 512 * nn2:ff * D + 512 * nn2 + 512],
                                start=(ff == 0), stop=(ff == 1))
                        nc.vector.tensor_copy(
                            ostage[:, 512 * nn2:512 * nn2 + 512], o[:])
                    nc.sync.dma_start(
                        out_d[b, 512 * j + 128 * g:512 * j + 128 * g + 128, :],
                        ostage[:])
                    drain_one()

    nc.compile()
    return nc


def make_in_maps(x, W_qkv, W_proj):
    bf16 = ml_dtypes.bfloat16
    tri = np.triu(np.ones((128, 128), dtype=np.float32)).astype(bf16)
    in_maps = []
    for c in range(NC):
        bg, hg = c // 4, c % 4
        xb = np.asarray(x[2 * bg:2 * bg + 2], dtype=np.float32)
        # xt[b, t5, dk, p, c] = x[b, t5*512 + c, dk*128 + p]
        xt = xb.reshape(NB, NT5, 512, NDK, 128).transpose(0, 1, 3, 4, 2)
        xt = np.ascontiguousarray(xt).astype(bf16)
        wq = np.concatenate(
            [W_qkv[:, 256 * hg:256 * hg + 256],
             W_qkv[:, 1024 + 256 * hg:1024 + 256 * hg + 256],
             W_qkv[:, 2048 + 256 * hg:2048 + 256 * hg + 256]], axis=1)
        # w_sb[p, dk*WCOL + col] = wq[dk*128 + p, col]
        wqs = np.ascontiguousarray(
            wq.reshape(NDK, 128, WCOL).transpose(1, 0, 2).reshape(128, NDK * WCOL)
        ).astype(bf16)
        wp = W_proj[256 * hg:256 * hg + 256, :]
        wps = np.ascontiguousarray(
            wp.reshape(2, 128, D).transpose(1, 0, 2).reshape(128, 2 * D)
        ).astype(bf16)
        in_maps.append({"xt": xt, "wqkv": wqs, "wproj": wps, "tri": tri})
    return in_maps


def kernel(x, W_qkv, W_proj):
    x = np.asarray(x, dtype=np.float32)
    W_qkv = np.asarray(W_qkv, dtype=np.float32)
    W_proj = np.asarray(W_proj, dtype=np.float32)
    nc = build()
    res = run_bass_kernel_spmd(nc, make_in_maps(x, W_qkv, W_proj), list(range(NC)))
    out = np.zeros((B, T, D), dtype=np.float64)
    for c in range(NC):
        bg = c // 4
        out[2 * bg:2 * bg + 2] += res.results[c]["out"].astype(np.float64)
    return out.astype(np.float32)


# revision 22
# speedup vs baseline: 1.0532x; 1.0532x over previous
"""Causal self-attention (B=4, T=2048, D=1024, H=16) on 8 TRN2 NeuronCores.

Sharding: tensor-parallel over 4 head-groups x data-parallel over 2 batch-groups.
Core c handles batches [2*(c//4), 2*(c//4)+2) and heads [4*(c%4), 4*(c%4)+4).
Each core computes a partial output projection (its 256 feature rows of W_proj);
the host sums the 4 head-group partials per batch group.

Design notes (v3):
- All matmul operands bf16 (PSUM accumulation fp32). Enables Fast Weight Load
  (LDWEIGHTS 97ns -> fully hidden under 216ns matmul streams) and halves SBUF
  traffic. rel-err lands ~4e-3 against the 2e-2 budget.
- x is transposed/packed on the host; x^T tiles DMA straight into SBUF.
- S^T uses 2x row tiling: each head contracts over only 64 dims, so the two
  heads of a packed Q^T/K^T pair run concurrently in rows 0-63 / 64-127 of the
  PE array (tile_position (0,0)/(64,0)), writing the two bank-halves of one
  [128,1024] PSUM tile; exp covers both heads in one ACT op.
- The V stationary block for (key-tile, head) is [64 ones cols | 64 V dims]:
  the PV matmul emits the softmax denominator pre-broadcast into PSUM rows
  0-63 for free (ones cols must map to base partition 0: the custom-DVE
  reciprocal ignores a nonzero input base partition).
- Softmax skips max-subtraction (scores ~N(0,1)) so exp never overflows.
- The QKV production of the NEXT 512-token chunk / next batch is interleaved
  into the attention inner loops as work units drained between PV pairs: the
  attention S-phase is exp(ACT)-paced, so QKV chain matmuls fill the PE gaps.
  QKV chain PSUM tiles and projection PSUM tiles share one 2-bank pool;
  attention needs 6 more (4 for double-buffered S^T pair tiles, 2 for PV
  accumulators) = exactly the 8 PSUM banks.
"""
import functools
from contextlib import ExitStack

import numpy as np
import ml_dtypes

import concourse.bacc as bacc
import concourse.tile as tile
import concourse.mybir as mybir
from concourse.bass_utils import run_bass_kernel_spmd

F32 = mybir.dt.float32
BF16 = mybir.dt.bfloat16
EXP = mybir.ActivationFunctionType.Exp

B, T, D, H, HD = 4, 2048, 1024, 16, 64
NB, NH = 2, 4            # batches / heads per core
NC = 8
NT5 = T // 512           # 4  (512-token chunks)
NTT = T // 128           # 16 (128-token key tiles)
NDK = D // 128           # 8  (feature chunks of input dim)
WCOL = 768               # per-dk weight columns: Q(256) K(256) V(256)


@functools.lru_cache(maxsize=1)
def build():
    nc = bacc.Bacc("TRN2", target_bir_lowering=False, debug=False, num_devices=NC)
    xt_d = nc.dram_tensor("xt", [NB, NT5, NDK, 128, 512], BF16,
                          kind="ExternalInput").ap()
    wqkv_d = nc.dram_tensor("wqkv", [128, NDK * WCOL], BF16,
                            kind="ExternalInput").ap()
    wproj_d = nc.dram_tensor("wproj", [128, 2 * D], BF16,
                             kind="ExternalInput").ap()
    tri_d = nc.dram_tensor("tri", [128, 128], BF16, kind="ExternalInput").ap()
    out_d = nc.dram_tensor("out", [NB, T, D], BF16, kind="ExternalOutput").ap()

    with tile.TileContext(nc) as tc, ExitStack() as ctx:
        const = ctx.enter_context(tc.tile_pool(name="const", bufs=1))
        wpool = ctx.enter_context(tc.tile_pool(name="w", bufs=1))
        actv = ctx.enter_context(tc.tile_pool(name="actv", bufs=1))
        xin_pool = ctx.enter_context(tc.tile_pool(name="xin", bufs=3))
        pP = ctx.enter_context(tc.tile_pool(name="pP", bufs=12))
        ytp = ctx.enter_context(tc.tile_pool(name="ytp", bufs=2))
        ost_pool = ctx.enter_context(tc.tile_pool(name="ost", bufs=2))
        rbp = ctx.enter_context(tc.tile_pool(name="rbp", bufs=2))
        # PSUM: pao (QKV chains + proj) 2 banks, psS 4 banks, psY 2 banks
        pao = ctx.enter_context(tc.tile_pool(name="pao", bufs=2, space="PSUM"))
        psS_pool = ctx.enter_context(
            tc.tile_pool(name="psS", bufs=2, space="PSUM"))
        psY_pool = ctx.enter_context(
            tc.tile_pool(name="psY", bufs=1, space="PSUM"))

        w_sb = wpool.tile([128, NDK * WCOL], BF16)
        wv8 = w_sb.rearrange("p (a c) -> p a c", a=NDK)
        wp_sb = wpool.tile([128, 2 * D], BF16)
        tri = const.tile([128, 128], BF16)          # tri[k,q] = 1.0 iff q >= k

        # per-batch double-buffered activation tiles
        qts = [[actv.tile([128, T], BF16, tag=f"qt{cc}", name=f"qt{cc}_{b}",
                          bufs=2) for cc in range(2)] for b in range(NB)]
        kts = [[actv.tile([128, T], BF16, tag=f"kt{cc}", name=f"kt{cc}_{b}",
                          bufs=2) for cc in range(2)] for b in range(NB)]
        vsbs = [actv.tile([128, NTT * NH * 128], BF16, tag="v", name=f"v_{b}",
                          bufs=2) for b in range(NB)]
        v128s = [v.rearrange("p (n c) -> p n c", c=128) for v in vsbs]

        # ---------- QKV production work units ----------
        # unit = (b, t5, thunk). B(b, j) requires all units with marker
        # (b', t5') <= (b, j) drained; the rest drain opportunistically in
        # attention PE gaps.
        def mk_dma_unit(b, t5):
            def f():
                xa = xin_pool.tile([128, NDK * 512], BF16, tag="xa",
                                   name=f"xa{b}_{t5}")
                xav = xa.rearrange("p (a c) -> p a c", a=NDK)
                for dk in range(NDK):
                    nc.sync.dma_start(xav[:, dk], xt_d[b, t5, dk])
                xas[(b, t5)] = xa
                if t5 == 0:
                    # ones for the denominator cols; split so the first V
                    # evacuation only waits on the first quarter
                    for q in range(4):
                        nc.gpsimd.memset(
                            vsbs[b][:, q * 2048:(q + 1) * 2048], 1.0)
            return f

        def mk_qk_unit(b, t5, kind, cc):
            def f():
                xa = xas[(b, t5)]
                dst = (qts if kind == 0 else kts)[b][cc]
                ps = pao.tile([128, 512], F32, tag="ps",
                              name=f"{'qk'[kind]}{b}{t5}{cc}")
                base = 256 * kind + cc * 128
                for dk in range(NDK):
                    nc.tensor.matmul(
                        ps[:],
                        w_sb[:, dk * WCOL + base:dk * WCOL + base + 128],
                        xa[:, dk * 512:dk * 512 + 512],
                        start=(dk == 0), stop=(dk == NDK - 1))
                nc.vector.tensor_copy(dst[:, t5 * 512:t5 * 512 + 512], ps[:])
            return f

        def mk_v_unit(b, t5, tt):
            def f():
                xa = xas[(b, t5)]
                ps = pao.tile([128, 256], F32, tag="ps", name=f"v{b}{t5}{tt}")
                for dk in range(NDK):
                    nc.tensor.matmul(
                        ps[:],
                        xa[:, dk * 512 + tt * 128:dk * 512 + tt * 128 + 128],
                        w_sb[:, dk * WCOL + 512:dk * WCOL + 768],
                        start=(dk == 0), stop=(dk == NDK - 1))
                ti = t5 * 4 + tt
                nc.vector.tensor_copy(
                    v128s[b][:, ti * NH:ti * NH + NH, 64:128],
                    ps[:].rearrange("p (n c) -> p n c", c=64))
            return f

        xas = {}
        units = []
        for b in range(NB):
            for t5 in range(NT5):
                units.append((b, t5, mk_dma_unit(b, t5)))
                for cc in range(2):
                    units.append((b, t5, mk_qk_unit(b, t5, 0, cc)))
                for cc in range(2):
                    units.append((b, t5, mk_qk_unit(b, t5, 1, cc)))
                for tt in range(4):
                    units.append((b, t5, mk_v_unit(b, t5, tt)))

        state = {"u": 0}

        def drain_until(b, t5):
            while state["u"] < len(units):
                ub, ut5, f = units[state["u"]]
                if (ub, ut5) > (b, t5):
                    return
                f()
                state["u"] += 1

        def drain_one():
            if state["u"] < len(units):
                units[state["u"]][2]()
                state["u"] += 1

        # input DMAs: descriptor generation costs ~0.6us per DMA per queue,
        # so spread the startup DMAs across idle queue engines; weight chunk
        # dk=0 goes first so the first QKV chain can start ASAP
        wqv8 = wqkv_d.rearrange("p (a c) -> p a c", a=NDK)
        for dk in range(NDK):
            eng = (nc.scalar, nc.gpsimd)[dk % 2]
            eng.dma_start(wv8[:, dk], wqv8[:, dk])
        units[0][2]()
        state["u"] = 1
        nc.scalar.dma_start(tri[:], tri_d)
        nc.gpsimd.dma_start(wp_sb[:], wproj_d)

        # ---------- main loop: attention + projection ----------
        for b in range(NB):
            for j in range(NT5):
                drain_until(b, j)
                nk = 4 * j + 4
                offs = [128 * (i - 4 * j) if i > 4 * j else 0
                        for i in range(nk)]
                yt = [ytp.tile([128, 512], BF16, tag=f"yt{ff}",
                               name=f"yt{ff}_{b}{j}") for ff in range(2)]
                for cc in range(2):
                    qt, kt = qts[b][cc], kts[b][cc]
                    psY = [psY_pool.tile([128, 512], F32, tag=f"y{hh}",
                                         name=f"psY{hh}") for hh in range(2)]
                    Ps = []

                    def emit_pv(i):
                        off = offs[i]
                        for hh in range(2):
                            nc.tensor.matmul(
                                psY[hh][:, off:512],
                                v128s[b][:, i * NH + 2 * cc + hh, :],
                                Ps[i][:, hh * 512 + off:hh * 512 + 512],
                                start=(i == 0), stop=(i == nk - 1))

                    for i in range(nk):
                        off = offs[i]
                        s = psS_pool.tile([128, 1024], F32, tag="s",
                                          name=f"s{i % 2}")
                        # two heads concurrently via 2x row tiling
                        nc.tensor.matmul(
                            s[:, off:512],
                            kt[0:64, 128 * i:128 * i + 128],
                            qt[0:64, 512 * j + off:512 * (j + 1)],
                            start=True, stop=True, tile_position=(0, 0))
                        nc.tensor.matmul(
                            s[:, 512 + off:1024],
                            kt[64:128, 128 * i:128 * i + 128],
                            qt[64:128, 512 * j + off:512 * (j + 1)],
                            start=True, stop=True, tile_position=(64, 0))
                        P = pP.tile([128, 1024], BF16, tag="p",
                                    name=f"P{i % 8}")
                        if off == 0:
                            nc.scalar.activation(P[:], s[:], EXP, scale=0.125)
                        else:
                            nc.scalar.activation(P[:, off:512], s[:, off:512],
                                                 EXP, scale=0.125)
                            nc.scalar.activation(P[:, 512 + off:1024],
                                                 s[:, 512 + off:1024],
                                                 EXP, scale=0.125)
                        if i >= 4 * j:      # causal diagonal tile mask
                            nc.vector.tensor_mul(
                                P[:, off:off + 128],
                                P[:, off:off + 128], tri[:])
                            nc.vector.tensor_mul(
                                P[:, 512 + off:512 + off + 128],
                                P[:, 512 + off:512 + off + 128], tri[:])
                        Ps.append(P)
                        # chase S^T with PV 4 key-tiles behind (paired to
                        # halve PE tiling-mode switches), and fill remaining
                        # PE slack with a QKV work unit
                        if i >= 4 and i % 2 == 0:
                            emit_pv(i - 4)
                            emit_pv(i - 3)
                            drain_one()
                    for i in range(max(0, nk - 4), nk):
                        emit_pv(i)
                    # normalize: PSUM rows 0-63 hold the denominator
                    # (broadcast by the all-ones V cols), rows 64-127 = y^T
                    for hh in range(2):
                        rb = rbp.tile([64, 512], F32, tag=f"rb{hh}",
                                      name=f"rb{hh}")
                        nc.vector.reciprocal_approx_fast(
                            rb[:], psY[hh][0:64, :])
                        nc.vector.tensor_mul(
                            yt[cc][64 * hh:64 * hh + 64, :],
                            psY[hh][64:128, :], rb[:])
                    drain_one()
                # ---- output projection for this 512-token block ----
                for g in range(4):
                    ostage = ost_pool.tile([128, D], BF16, tag="og",
                                           name=f"og{g % 2}")
                    for nn2 in range(2):
                        o = pao.tile([128, 512], F32, tag="ps", name=f"o{nn2}")
                        for ff in range(2):
                            nc.tensor.matmul(
                                o[:],
                                yt[ff][:, 128 * g:128 * g + 128],
                                wp_sb[:, ff * D + 512 * nn2:ff * D + 512 * nn2 + 512],
                                start=(ff == 0), stop=(ff == 1))
                        nc.vector.tensor_copy(
                            ostage[:, 512 * nn2:512 * nn2 + 512], o[:])
                    nc.sync.dma_start(
                        out_d[b, 512 * j + 128 * g:512 * j + 128 * g + 128, :],
                        ostage[:])
                    drain_one()

    nc.compile()
    return nc


def make_in_maps(x, W_qkv, W_proj):
    bf16 = ml_dtypes.bfloat16
    tri = np.triu(np.ones((128, 128), dtype=np.float32)).astype(bf16)
    in_maps = []
    for c in range(NC):
        bg, hg = c // 4, c % 4
        xb = np.asarray(x[2 * bg:2 * bg + 2], dtype=np.float32)
        # xt[b, t5, dk, p, c] = x[b, t5*512 + c, dk*128 + p]
        xt = xb.reshape(NB, NT5, 512, NDK, 128).transpose(0, 1, 3, 4, 2)
        xt = np.ascontiguousarray(xt).astype(bf16)
        wq = np.concatenate(
            [W_qkv[:, 256 * hg:256 * hg + 256],
             W_qkv[:, 1024 + 256 * hg:1024 + 256 * hg + 256],
             W_qkv[:, 2048 + 256 * hg:2048 + 256 * hg + 256]], axis=1)
        # w_sb[p, dk*WCOL + col] = wq[dk*128 + p, col]
        wqs = np.ascontiguousarray(
            wq.reshape(NDK, 128, WCOL).transpose(1, 0, 2).reshape(128, NDK * WCOL)
        ).astype(bf16)
        wp = W_proj[256 * hg:256 * hg + 256, :]
        wps = np.ascontiguousarray(
            wp.reshape(2, 128, D).transpose(1, 0, 2).reshape(128, 2 * D)
        ).astype(bf16)
        in_maps.append({"xt": xt, "wqkv": wqs, "wproj": wps, "tri": tri})
    return in_maps


def kernel(x, W_qkv, W_proj):
    x = np.asarray(x, dtype=np.float32)
    W_qkv = np.asarray(W_qkv, dtype=np.float32)
    W_proj = np.asarray(W_proj, dtype=np.float32)
    nc = build()
    res = run_bass_kernel_spmd(nc, make_in_maps(x, W_qkv, W_proj), list(range(NC)))
    out = np.zeros((B, T, D), dtype=np.float64)
    for c in range(NC):
        bg = c // 4
        out[2 * bg:2 * bg + 2] += res.results[c]["out"].astype(np.float64)
    return out.astype(np.float32)
